# revision 57
# baseline (speedup 1.0000x reference)
"""Trainium2 Bass kernel for nn_Attention_72816875536915.

Multi-head attention (QKV proj + per-head RMSNorm + RoPE + softmax attention
+ output proj), tensor-parallel over heads across 8 NeuronCores.

Layout/dataflow (per core c, owning heads {2c, 2c+1}):
  Host prep: hidden pre-transposed to [D, B*S] bf16; Wq/Wk/Wv column shards
    (head dims permuted to [evens|odds] so RoPE pairs are contiguous column
    halves) in bf16; RoPE tables (g folded in) packed into one [B*S, 512]
    bf16 tensor; Wo rows permuted to the post-all-to-all dim order, bf16.
  Phase 1 (QKV+RMSNorm+RoPE): stream hidT blocks; QKV matmuls (bf16,
    hidT chunk stationary, weights moving); RMSNorm sums-of-squares via ACT
    Square+accum; RoPE on DVE in bf16; PE-transpose q^,k^ to [dim, token];
    K^T, Q^T, V all cached in SBUF bf16 for the whole run (no DRAM spill).
  Phase 2 (attention, hl outer): scores^T = kT.T @ qT per 128-key chunk;
    Exp on ACT (no max subtraction: RMS-normed q,k bound |score|<=sqrt(128));
    PE column-sums of probs (ones stationary) and V contraction accumulate
    in PSUM; normalize; write bf16 oT into the all-to-all buffers. Wo is
    preloaded to SBUF here (SP/DMA are otherwise idle through phase 2).
  Phase 3: three AllToAlls: hl=0 full-width (issued mid-phase-2, fully
    hidden), then hl=1 in two token-halves so the last collective launches
    before the hl=1 loop fully drains.
  Phase 4: y = attn_all^T.T @ Wo_perm from SBUF, two passes over the
    contraction (PSUM holds half the f32 output): pass A = hl=0 dims
    (depends only on the hidden AllToAll), partials parked in SBUF; pass B
    = hl=1 dims + add, so almost all matmul work is runnable before the
    last collective lands.
"""

import sys

sys.path.insert(0, "/opt/trn_rl_repo")

import math
import numpy as np

import concourse.mybir as mybir
import concourse.tile as tile
from concourse import bacc
from concourse.bass_utils import run_bass_kernel_spmd
from concourse.masks import make_identity

# Problem geometry (hardcoded per the harness contract).
B = 2
S = 2048
D = 2048
H = 16
HD = 128
NCORES = 8
HPC = H // NCORES          # heads per core
DLOC = HPC * HD            # local head dims per core
EPS = 1e-5

F32 = mybir.dt.float32
BF16 = mybir.dt.bfloat16
MULT = mybir.AluOpType.mult
ADD = mybir.AluOpType.add
SUB = mybir.AluOpType.subtract
AF = mybir.ActivationFunctionType

SB = 256                   # phase-1 token block (2 x 128)

last_run_info = {}


def build(s=S):
    bs = B * s
    rpc = bs // NCORES     # output rows (tokens) per core
    assert rpc % 128 == 0 and s % rpc == 0
    sqc = rpc              # phase-2 query chunk == one dest core's tokens
    n_blk = bs // SB
    n_ksub = s // 128
    inv_sqrt_hd = 1.0 / math.sqrt(HD)

    nc = bacc.Bacc(
        "TRN2", target_bir_lowering=False, debug=False, num_devices=NCORES
    )

    hidT = nc.dram_tensor("hidT", [D, bs], BF16, kind="ExternalInput")
    wq = nc.dram_tensor("wq", [D, DLOC], BF16, kind="ExternalInput")
    wk = nc.dram_tensor("wk", [D, DLOC], BF16, kind="ExternalInput")
    wv = nc.dram_tensor("wv", [D, DLOC], BF16, kind="ExternalInput")
    wo = nc.dram_tensor("wo", [D, D], BF16, kind="ExternalInput")
    tab8 = nc.dram_tensor("tab8", [bs, 8 * (HD // 2)], BF16, kind="ExternalInput")
    out = nc.dram_tensor("out", [rpc, D], F32, kind="ExternalOutput")

    with tile.TileContext(nc) as tc:
        with (
            tc.tile_pool(name="const", bufs=1) as const_pool,
            tc.tile_pool(name="cache", bufs=1) as cache_pool,
            tc.tile_pool(name="dram", bufs=1, space="DRAM") as dram_pool,
        ):
            ident_f32 = const_pool.tile([128, 128], F32)
            make_identity(nc, ident_f32)
            ident = const_pool.tile([128, 128], BF16)
            nc.vector.tensor_copy(ident[:], ident_f32[:])
            ones_b = const_pool.tile([128, 128], BF16)
            nc.gpsimd.memset(ones_b[:], 1.0)
            epsb = const_pool.tile([128, 1], F32)
            nc.gpsimd.memset(epsb[:], EPS)

            wqk_sb = const_pool.tile([128, D // 128, 2 * DLOC], BF16)
            wv_sb = const_pool.tile([128, D // 128, DLOC], BF16)

            def load_weights(ck4):
                # q|k packed into one tile so each contraction chunk needs
                # 2 matmuls instead of 3; chunked so the first QKV matmuls
                # can start early
                for wdr, wsb, c0 in (
                    (wq, wqk_sb, 0),
                    (wk, wqk_sb, DLOC),
                    (wv, wv_sb, 0),
                ):
                    nc.sync.dma_start(
                        wsb[:, ck4 * 4 : (ck4 + 1) * 4, c0 : c0 + DLOC],
                        wdr.ap()[ck4 * 512 : (ck4 + 1) * 512, :].rearrange(
                            "(o p) n -> p o n", p=128
                        ),
                    )

            kT_all = cache_pool.tile([128, HPC, bs], BF16)
            qT_all = cache_pool.tile([128, HPC, bs], BF16)
            v_all = cache_pool.tile([128, bs // 128, DLOC], BF16)

            # hl=0: one full-width a2a; hl=1: two token-half a2a's so the
            # last collective starts before the full hl=1 loop finishes.
            a2a_in0 = dram_pool.tile([NCORES * HD, rpc], BF16, name="a2a_in0")
            a2a_out0 = dram_pool.tile([NCORES * HD, rpc], BF16, name="a2a_out0")
            a2a_in1 = [
                dram_pool.tile([NCORES * HD, rpc // 2], BF16, name=f"a2a_in1{t}")
                for t in range(2)
            ]
            a2a_out1 = [
                dram_pool.tile([NCORES * HD, rpc // 2], BF16, name=f"a2a_out1{t}")
                for t in range(2)
            ]

            # ============ Phase 1: QKV + RMSNorm + RoPE ============
            with (
                tc.tile_pool(name="p1", bufs=2) as p1,
                tc.tile_pool(name="p1s", bufs=3) as p1s,
                tc.tile_pool(name="psqkv", bufs=2, space="PSUM") as psqkv,
                tc.tile_pool(name="pstp", bufs=2, space="PSUM") as pstp,
            ):
                for blk in range(n_blk):
                    r0 = blk * SB
                    hT = p1.tile([128, D // 128, SB], BF16, tag="hT")
                    nc.sync.dma_start(
                        hT[:],
                        hidT.ap()[:, r0 : r0 + SB].rearrange(
                            "(o p) t -> p o t", p=128
                        ),
                    )
                    tab = p1.tile([128, SB // 128, 512], BF16, tag="tab")
                    nc.sync.dma_start(
                        tab[:],
                        tab8.ap()[r0 : r0 + SB, :].rearrange(
                            "(i p) f -> p i f", p=128
                        ),
                    )
                    if blk == 0:
                        for ck4 in range(4):
                            load_weights(ck4)
                    for i in range(SB // 128):
                        pqk = psqkv.tile([128, 2 * DLOC], F32, tag="pqk", name="pqk")
                        pv = psqkv.tile([128, DLOC], F32, tag="pv")
                        pq = pqk[:, 0:DLOC]
                        pk = pqk[:, DLOC : 2 * DLOC]
                        for ck in range(D // 128):
                            fl = dict(start=(ck == 0), stop=(ck == D // 128 - 1))
                            lt = hT[:, ck, i * 128 : (i + 1) * 128]
                            nc.tensor.matmul(pqk[:], lt, wqk_sb[:, ck, :], **fl)
                            nc.tensor.matmul(pv[:], lt, wv_sb[:, ck, :], **fl)
                        nc.scalar.activation(
                            v_all[:, (r0 // 128) + i, :], pv[:], AF.Copy
                        )

                        for ps_t, toff, dstT in ((pq, 0, qT_all), (pk, 4, kT_all)):
                            nat = p1s.tile([128, DLOC], BF16, tag="nat")
                            nc.scalar.activation(nat[:], ps_t, AF.Copy)
                            ms = p1s.tile([128, HPC], F32, tag="ms")
                            sqs = p1s.tile([128, HD], BF16, tag="sqs")
                            for hl in range(HPC):
                                nc.scalar.activation(
                                    sqs[:],
                                    ps_t[:, hl * HD : (hl + 1) * HD],
                                    AF.Square,
                                    accum_out=ms[:, hl : hl + 1],
                                )
                            rms = p1s.tile([128, HPC], F32, tag="rms")
                            nc.scalar.activation(
                                rms[:], ms[:], AF.Sqrt, bias=epsb[:], scale=1.0 / HD
                            )
                            rinv = p1s.tile([128, HPC], F32, tag="rinv")
                            nc.vector.reciprocal(rinv[:], rms[:])
                            hat = p1s.tile([128, DLOC], BF16, tag="hat")
                            t1 = p1s.tile([128, 64], BF16, tag="t1")
                            t2 = p1s.tile([128, 64], BF16, tag="t2")
                            xn = p1s.tile([128, DLOC], BF16, tag="xn")
                            for hl in range(HPC):
                                c0 = hl * HD
                                nc.vector.tensor_scalar_mul(
                                    xn[:, c0 : c0 + HD],
                                    nat[:, c0 : c0 + HD],
                                    rinv[:, hl : hl + 1],
                                )
                                xr = xn[:, c0 : c0 + 64]
                                xi = xn[:, c0 + 64 : c0 + HD]
                                cr = tab[:, i, toff * 64 : toff * 64 + 64]
                                sr = tab[:, i, (toff + 1) * 64 : (toff + 1) * 64 + 64]
                                ci = tab[:, i, (toff + 2) * 64 : (toff + 2) * 64 + 64]
                                si = tab[:, i, (toff + 3) * 64 : (toff + 3) * 64 + 64]
                                # even half: xr*cr - xi*sr
                                nc.vector.tensor_tensor(t1[:], xr, cr, MULT)
                                nc.vector.tensor_tensor(t2[:], xi, sr, MULT)
                                nc.vector.tensor_tensor(
                                    hat[:, c0 : c0 + 64], t1[:], t2[:], SUB
                                )
                                # odd half: xr*si + xi*ci
                                nc.vector.tensor_tensor(t1[:], xr, si, MULT)
                                nc.vector.tensor_tensor(t2[:], xi, ci, MULT)
                                nc.vector.tensor_tensor(
                                    hat[:, c0 + 64 : c0 + HD], t1[:], t2[:], ADD
                                )
                            for hl in range(HPC):
                                ptp = pstp.tile([128, 128], BF16, tag="ptp")
                                nc.tensor.transpose(
                                    ptp[:], hat[:, hl * HD : (hl + 1) * HD], ident[:]
                                )
                                nc.vector.tensor_copy(
                                    dstT[:, hl, r0 + i * 128 : r0 + (i + 1) * 128],
                                    ptp[:],
                                )

            # attention-output tiles for phase 4 (loads emitted in/after ph2)
            aT0 = [
                cache_pool.tile([128, NCORES, rpc // 2], BF16, name=f"aT0{t}")
                for t in range(2)
            ]
            aT1 = [
                cache_pool.tile([128, NCORES, rpc // 2], BF16, name=f"aT1{t}")
                for t in range(2)
            ]

            # Wo preload (SP/HWDGE are idle through phase 2)
            wo_sb = cache_pool.tile([128, D // 128, D], BF16)
            for ck4 in range(4):
                nc.sync.dma_start(
                    wo_sb[:, ck4 * 4 : (ck4 + 1) * 4, :],
                    wo.ap()[ck4 * 512 : (ck4 + 1) * 512, :].rearrange(
                        "(o p) n -> p o n", p=128
                    ),
                )

            # ============ Phase 2: attention + per-head AllToAll ============
            with (
                tc.tile_pool(name="p2", bufs=3) as p2,
                tc.tile_pool(name="p2s", bufs=2) as p2s,
                tc.tile_pool(name="ps2", bufs=3, space="PSUM") as ps2,
                tc.tile_pool(name="ps2v", bufs=2, space="PSUM") as ps2v,
                tc.tile_pool(name="ps2s", bufs=2, space="PSUM") as ps2s,
            ):
                def attn_block(hl, b, q_off, w, dst_ap):
                    """softmax(qT[:, q_off:q_off+w] scores) @ V -> oT -> dst.

                    Narrow (w<512) score chunks come in pairs sharing one
                    PSUM tile so Exp runs once per pair, halving ACT
                    instruction overhead and keeping ACT off the critical
                    path."""
                    sums_t = ps2s.tile([128, w], F32, tag="sums", bufs=2, name="sums_t")
                    po_t = ps2v.tile([128, w], F32, tag="po", bufs=2, name="po_t")
                    sums_b = sums_t[:]
                    po = po_t[:]
                    nj = 1 if w >= 512 else 2   # exp-batch narrow chunks
                    for jp in range(n_ksub // nj):
                        sc_t = ps2.tile([128, nj, w], F32, tag="sc", bufs=4, name="sc_t")
                        sc = sc_t[:]
                        for jj in range(nj):
                            j = jp * nj + jj
                            nc.tensor.matmul(
                                sc[:, jj, :],
                                kT_all[:, hl, b * s + j * 128 : b * s + (j + 1) * 128],
                                qT_all[:, hl, q_off : q_off + w],
                                start=True,
                                stop=True,
                            )
                        pr = p2.tile([128, nj, w], BF16, tag="pr", bufs=3)
                        nc.scalar.activation(pr[:], sc, AF.Exp, scale=inv_sqrt_hd)
                        for jj in range(nj):
                            j = jp * nj + jj
                            jf = dict(start=(j == 0), stop=(j == n_ksub - 1))
                            nc.tensor.matmul(sums_b, ones_b[:], pr[:, jj, :], **jf)
                            nc.tensor.matmul(
                                po,
                                v_all[:, (b * s) // 128 + j, hl * HD : (hl + 1) * HD],
                                pr[:, jj, :],
                                **jf,
                            )
                    recb = p2s.tile([128, w], F32, tag="recb", bufs=6)
                    nc.vector.reciprocal_approx_fast(recb[:], sums_b)
                    oT = p2s.tile([128, w], BF16, tag="oT", bufs=6)
                    nc.vector.tensor_tensor(oT[:], po, recb[:], MULT)
                    nc.sync.dma_start(dst_ap, oT[:])

                grp = [list(range(NCORES))]
                for b in range(B):
                    for qc in range(s // sqc):
                        q_off = b * s + qc * sqc
                        dest = q_off // rpc
                        attn_block(
                            0, b, q_off, sqc,
                            a2a_in0[dest * HD : (dest + 1) * HD, :],
                        )
                nc.gpsimd.collective_compute(
                    "AllToAll", mybir.AluOpType.bypass, replica_groups=grp,
                    ins=[a2a_in0[:].opt()], outs=[a2a_out0[:].opt()],
                )
                for th in range(2):
                    hw = sqc // 2
                    for b in range(B):
                        for qc in range(s // sqc):
                            q_off = b * s + qc * sqc + th * hw
                            dest = (b * s + qc * sqc) // rpc
                            attn_block(
                                1, b, q_off, hw,
                                a2a_in1[th][dest * HD : (dest + 1) * HD, :],
                            )
                            if th == 1 and b == 0 and qc == 0:
                                # prefetch a2a_out0 on SP right after the
                                # first th=1 oT write: coll0's semaphore has
                                # long fired, and later oT writes aren't
                                # ready yet, so these loads hide in the gap.
                                for t2 in range(2):
                                    nc.sync.dma_start(
                                        aT0[t2][:],
                                        a2a_out0[
                                            :, t2 * (rpc // 2) : (t2 + 1) * (rpc // 2)
                                        ].rearrange("(o p) t -> p o t", p=128),
                                    )
                    nc.gpsimd.collective_compute(
                        "AllToAll", mybir.AluOpType.bypass, replica_groups=grp,
                        ins=[a2a_in1[th][:].opt()], outs=[a2a_out1[th][:].opt()],
                    )

            # ============ Phase 4: output projection ============
            with (
                tc.tile_pool(name="p4y", bufs=4) as p4y,
                tc.tile_pool(name="ps4", bufs=8, space="PSUM") as ps4,
            ):
                n_tok = rpc // 128   # 128-token output row groups
                # tile_wait_until keeps the scheduler from hoisting these
                # waits above the th=1 oT writes that gate the last
                # collective (SP would sit blocked on the a2a semaphore).
                for th in range(2):
                    with tc.tile_wait_until(0.36 * (s / S)):
                        nc.sync.dma_start(
                            aT1[th][:],
                            a2a_out1[th][:].rearrange("(o p) t -> p o t", p=128),
                        )
                aTs = [aT0, aT1]

                def stat(ckh, c, i):
                    # stationary [128 dims, 128 tokens] for contraction chunk
                    return aTs[ckh][i // 2][:, c, (i % 2) * 128 : (i % 2) * 128 + 128]

                nq = D // 512        # output column quarters
                # Two-pass over the contraction: pass A (hl=0 dims, aT0 only)
                # accumulates in PSUM and parks partials in SBUF; pass B
                # (hl=1 dims) accumulates in PSUM and adds the partials.
                # The full [rpc, D] f32 output is 2x PSUM capacity, and this
                # split maximizes the work runnable before the last a2a lands.
                ya = {}
                for i in range(n_tok):
                    for q in range(nq):
                        pya = ps4.tile([128, 512], F32, tag="pya", bufs=4)
                        for c in range(NCORES):
                            nc.tensor.matmul(
                                pya[:],
                                stat(0, c, i),
                                wo_sb[:, c, q * 512 : (q + 1) * 512],
                                start=(c == 0),
                                stop=(c == NCORES - 1),
                            )
                        t = p4y.tile(
                            [128, 512], F32, tag="ya", bufs=16, name=f"ya_{q}_{i}"
                        )
                        nc.scalar.activation(t[:], pya[:], AF.Copy)
                        ya[(q, i)] = t

                for i in range(n_tok):
                    for q in range(nq):
                        pyb = ps4.tile([128, 512], F32, tag="pyb", bufs=4)
                        for c in range(NCORES):
                            nc.tensor.matmul(
                                pyb[:],
                                stat(1, c, i),
                                wo_sb[:, NCORES + c, q * 512 : (q + 1) * 512],
                                start=(c == 0),
                                stop=(c == NCORES - 1),
                            )
                        y_sb = p4y.tile([128, 512], F32, tag="y_sb", bufs=4)
                        nc.vector.tensor_tensor(
                            y_sb[:], pyb[:], ya[(q, i)][:], ADD
                        )
                        nc.sync.dma_start(
                            out.ap()[
                                i * 128 : (i + 1) * 128, q * 512 : (q + 1) * 512
                            ],
                            y_sb[:],
                        )

    nc.compile()
    return nc


_PERM = np.concatenate([np.arange(0, HD, 2), np.arange(1, HD, 2)])


def shard_inputs(hidden_states, freqs_cos, freqs_sin, Wq, Wk, Wv, Wo, gq, gk, s=S):
    """Host-side prep: per-core input dicts (all matmul operands in bf16)."""
    import ml_dtypes

    bf = ml_dtypes.bfloat16
    bs = B * s
    hidT = np.ascontiguousarray(
        hidden_states.reshape(bs, D).astype(np.float32).T.astype(bf)
    )
    cos = freqs_cos.reshape(bs, HD // 2).astype(np.float32)
    sin = freqs_sin.reshape(bs, HD // 2).astype(np.float32)
    gq = gq.astype(np.float32)
    gk = gk.astype(np.float32)
    # packed RoPE tables, g folded in: [cqr sqr cqi sqi ckr skr cki ski]
    tab8 = np.concatenate(
        [
            cos * gq[_PERM[: HD // 2]],
            sin * gq[_PERM[: HD // 2]],
            cos * gq[_PERM[HD // 2 :]],
            sin * gq[_PERM[HD // 2 :]],
            cos * gk[_PERM[: HD // 2]],
            sin * gk[_PERM[: HD // 2]],
            cos * gk[_PERM[HD // 2 :]],
            sin * gk[_PERM[HD // 2 :]],
        ],
        axis=1,
    ).astype(bf)
    tab8 = np.ascontiguousarray(tab8)
    # Wo rows permuted to post-a2a order: [head 2c+0 blocks | head 2c+1 blocks]
    rperm = np.concatenate(
        [
            np.arange((2 * c + hl) * HD, (2 * c + hl + 1) * HD)
            for hl in range(HPC)
            for c in range(NCORES)
        ]
    )
    wo_perm = np.ascontiguousarray(Wo.astype(np.float32)[rperm].astype(bf))
    in_maps = []
    for c in range(NCORES):
        cols = []
        for hl in range(HPC):
            g = HPC * c + hl
            cols.extend((g * HD + _PERM).tolist())
        cols = np.array(cols)
        vcols = np.arange(HPC * c * HD, (HPC * c + HPC) * HD)
        m = {
            "hidT": hidT,
            "wq": np.ascontiguousarray(Wq[:, cols].astype(np.float32).astype(bf)),
            "wk": np.ascontiguousarray(Wk[:, cols].astype(np.float32).astype(bf)),
            "wv": np.ascontiguousarray(Wv[:, vcols].astype(np.float32).astype(bf)),
            "wo": wo_perm,
            "tab8": tab8,
        }
        in_maps.append(m)
    return in_maps


_NC_CACHE = {}


def kernel(hidden_states, freqs_cos, freqs_sin, Wq, Wk, Wv, Wo, gq, gk):
    inputs = dict(
        hidden_states=np.asarray(hidden_states),
        freqs_cos=np.asarray(freqs_cos),
        freqs_sin=np.asarray(freqs_sin),
        Wq=np.asarray(Wq),
        Wk=np.asarray(Wk),
        Wv=np.asarray(Wv),
        Wo=np.asarray(Wo),
        gq=np.asarray(gq),
        gk=np.asarray(gk),
    )
    if S not in _NC_CACHE:
        _NC_CACHE[S] = build(S)
    nc = _NC_CACHE[S]
    in_maps = shard_inputs(**inputs, s=S)
    res = run_bass_kernel_spmd(nc, in_maps, core_ids=list(range(NCORES)))
    last_run_info["exec_time_ns"] = res.exec_time_ns
    y = np.concatenate([res.results[c]["out"] for c in range(NCORES)], axis=0)
    return y.reshape(B, S, D).astype(np.float32)
